# revision 1
# baseline (speedup 1.0000x reference)
"""Trainium2 Bass kernel for nn_Consistent_loss_up_2 (scatter_memory).

Reference computation:
    bins = round(up*50+110) clipped to [0,255]; mask = up >= 0.0235
    scatter-max over i into up2left/up2right[k, 0, j, bin]:
        i > 128:  value (i-128)/60  -> up2right
        i <= 128: value (128-i)/60  -> up2left
    loss = mean(|up2right-right| masked) + mean(|up2left-left| masked)
    where masked = (d < 0.2) & (map != 0)

Key structure exploited:
  * only bins 111..160 (50 of 256) are reachable -> tables are [j, 100]
    (left at cols [0,50), right at [50,100))
  * scatter values are monotone in i, so scatter-max == overwrite-scatter
    in the right stream order (left: i descending, right: i ascending);
    gpsimd local_scatter is last-write-wins (verified on HW)
  * the left-stream reversal is folded into the PE transpose by using an
    anti-diagonal "identity" matrix
  * final output is a scalar: each core returns [128,2] partial sums

Sharding: data-parallel over batch B=128 across 8 cores (16 each).

Engine budget per core (cost model): DVE ~32us, ACT ~21us, Pool ~16us,
PE ~9us, DMA ~6MB.
"""

import numpy as np

from concourse import bacc, mybir, tile
from concourse.bass_utils import run_bass_kernel_spmd

B, H, W = 128, 256, 256
NCORES = 8
KPC = B // NCORES  # batches per core = 16
NBIN = 50          # reachable bins: 111..160
OFF = 1024.0       # table-value offset so empty bins auto-fail the d<0.2 test
R23 = 8388608.0    # 2^23: round-to-nearest-even trick
MASK_SUB = 4000.0  # pushes masked points' indices negative
TBLW = 100         # per-k table width: left at [0,50), right at [50,100)
REFW = TBLW * KPC  # staged width per j-tile = 1600

_cache = {}


def _build_bass():
    nc = bacc.Bacc("TRN2", target_bir_lowering=False)
    f32, i16 = mybir.dt.float32, mybir.dt.int16
    Alu = mybir.AluOpType
    Act = mybir.ActivationFunctionType

    up_in = nc.dram_tensor("up_in", [KPC * H, W], f32, kind="ExternalInput")
    refs_in = nc.dram_tensor("refs_in", [W, REFW], f32, kind="ExternalInput")
    vee_in = nc.dram_tensor("vee_in", [128, 256], i16, kind="ExternalInput")
    ro1_in = nc.dram_tensor("ro1_in", [128, 1], f32, kind="ExternalInput")
    off_in = nc.dram_tensor("off_in", [128, 1], f32, kind="ExternalInput")
    ident_in = nc.dram_tensor("ident_in", [128, 128], f32, kind="ExternalInput")
    antid_in = nc.dram_tensor("antid_in", [128, 128], f32, kind="ExternalInput")
    out = nc.dram_tensor("out", [128, 2], f32, kind="ExternalOutput")

    with tile.TileContext(nc) as tc:
        with (
            tc.tile_pool(name="const", bufs=1) as constp,
            tc.tile_pool(name="stage", bufs=1) as stagep,
            tc.tile_pool(name="work", bufs=3) as workp,
            tc.tile_pool(name="psum", bufs=4, space="PSUM") as psump,
            tc.tile_pool(name="loss", bufs=1) as lossp,
        ):
            vee = constp.tile([128, 256], i16)
            nc.sync.dma_start(vee[:], vee_in[:])
            ro1 = constp.tile([128, 1], f32)
            nc.sync.dma_start(ro1[:], ro1_in[:])
            offc = constp.tile([128, 1], f32)
            nc.sync.dma_start(offc[:], off_in[:])
            ident = constp.tile([128, 128], f32)
            nc.sync.dma_start(ident[:], ident_in[:])
            antid = constp.tile([128, 128], f32)
            nc.sync.dma_start(antid[:], antid_in[:])

            refs_sb = []
            tbl = []
            for jt in range(2):
                r = stagep.tile([128, REFW], f32, tag=f"refs{jt}")
                nc.scalar.dma_start(r[:], refs_in[jt * 128:(jt + 1) * 128, :])
                refs_sb.append(r)
                tbl.append(
                    stagep.tile([128, REFW], i16, tag=f"tbl{jt}", name=f"tbl{jt}")
                )

            for k in range(KPC):
                # one DMA per k: partition p <- rows (256k+p, 256k+128+p)
                ut = workp.tile([128, 2, W], f32, tag="ut")
                src = up_in[k * H:(k + 1) * H, :].rearrange(
                    "(h p) w -> p h w", h=2
                )
                nc.sync.dma_start(ut[:], src)
                utm = ut[:].rearrange("p h w -> p (h w)")

                # f = 50*u + 110 on ScalarE (both halves at once)
                fm = workp.tile([128, 2 * W], f32, tag="fm")
                nc.scalar.activation(
                    fm[:], utm, Act.Copy, bias=110.0, scale=50.0
                )
                # mask term on DVE (both halves at once)
                mk = workp.tile([128, 2 * W], f32, tag="mk")
                nc.vector.tensor_scalar(
                    mk[:], utm, 0.0235, MASK_SUB, op0=Alu.is_lt, op1=Alu.mult
                )
                # RNE rounding + per-half bin offset
                rbm = workp.tile([128, 2 * W], f32, tag="rbm")
                nc.vector.tensor_scalar(
                    rbm[:, 0:W], fm[:, 0:W], R23, R23 + 111.0,
                    op0=Alu.add, op1=Alu.subtract,
                )
                nc.vector.tensor_scalar(
                    rbm[:, W:2 * W], fm[:, W:2 * W], R23, ro1[:, :],
                    op0=Alu.add, op1=Alu.subtract,
                )
                ixm = workp.tile([128, 2 * W], f32, tag="ixm")
                nc.vector.tensor_tensor(
                    out=ixm[:], in0=rbm[:], in1=mk[:], op=Alu.subtract
                )

                for jt in range(2):
                    js = slice(jt * 128, (jt + 1) * 128)
                    ps = psump.tile([128, 256], f32, tag=f"ps{jt}", space="PSUM")
                    # anti-diagonal identity reverses columns: col n <-> i=127-n
                    nc.tensor.transpose(ps[:, 0:128], ixm[:, js], antid[:])
                    nc.tensor.transpose(
                        ps[:, 128:256], ixm[:, 256 + jt * 128:256 + (jt + 1) * 128],
                        ident[:],
                    )
                    st = workp.tile([128, 256], i16, tag=f"st{jt}")
                    nc.scalar.activation(st[:], ps[:], Act.Copy)

                    nc.gpsimd.local_scatter(
                        tbl[jt][:, k * TBLW:(k + 1) * TBLW],
                        vee[:],
                        st[:],
                        channels=128,
                        num_elems=TBLW,
                        num_idxs=256,
                    )

            for jt in range(2):
                e = lossp.tile([128, REFW], f32, tag="e")
                nc.vector.scalar_tensor_tensor(
                    e[:], refs_sb[jt][:], 60.0, tbl[jt][:],
                    op0=Alu.mult, op1=Alu.subtract,
                )
                a = lossp.tile([128, REFW], f32, tag="a")
                nc.scalar.activation(
                    a[:], e[:], Act.Abs, bias=offc[:, :], scale=1.0
                )
                cm = lossp.tile([128, REFW], f32, tag="cm")
                nc.vector.tensor_scalar(cm[:], a[:], 12.0, None, op0=Alu.is_lt)
                m = lossp.tile([128, REFW], f32, tag="m")
                nc.vector.tensor_tensor(
                    out=m[:], in0=a[:], in1=cm[:], op=Alu.mult
                )
                junk = lossp.tile([128, REFW], f32, tag="junk")
                part = lossp.tile([128, 1], f32, tag=f"part{jt}")
                nc.scalar.activation(
                    junk[:], m[:], Act.Copy, accum_out=part[:]
                )
                nc.scalar.dma_start(out[:, jt:jt + 1], part[:])

    nc.compile()
    return nc


def _host_constants():
    # scatter data stream values:
    #   pos n in [0,128): i = 127-n (left)  -> value (128-i)+OFF = n+1+OFF
    #   pos n in [128,256): i = n (right)   -> value (i-128)+OFF = n-128+OFF
    n = np.arange(256)
    vee = np.where(n < 128, n + 1, n - 128).astype(np.int16) + np.int16(OFF)
    vee = np.ascontiguousarray(np.broadcast_to(vee, (128, 256)))

    # per-partition subtrahend for tile1 (i = 128+p):
    #   p=0 is i=128 -> always skip; p>=1 -> right table at +50
    ro1 = np.full((128, 1), R23 + 61.0, np.float32)
    ro1[0, 0] = R23 + 311.0

    ident = np.eye(128, dtype=np.float32)
    antid = np.ascontiguousarray(ident[::-1, :])
    return vee, ro1, ident, antid


def _prep_refs(left, right):
    """[256, REFW] per core: row j (jt*128+p), col k*100+[0:50)=left slice,
    k*100+50+[0:50)=right slice."""
    lft = left[:, 0, :, 111:161]   # [B, W, 50]
    rgt = right[:, 0, :, 111:161]
    refs = np.zeros((NCORES, 2, 128, KPC, TBLW), np.float32)
    lv = lft.reshape(NCORES, KPC, 2, 128, NBIN).transpose(0, 2, 3, 1, 4)
    rv = rgt.reshape(NCORES, KPC, 2, 128, NBIN).transpose(0, 2, 3, 1, 4)
    refs[..., 0:NBIN] = lv
    refs[..., NBIN:2 * NBIN] = rv
    return refs.reshape(NCORES, W, REFW)


def make_in_maps(up, left, right):
    up = np.asarray(up, np.float32)
    left = np.asarray(left, np.float32)
    right = np.asarray(right, np.float32)
    vee, ro1, ident, antid = _host_constants()
    refs = _prep_refs(left, right)
    in_maps = []
    for c in range(NCORES):
        upc = np.ascontiguousarray(
            up[c * KPC:(c + 1) * KPC, 0].reshape(KPC * H, W)
        )
        in_maps.append({
            "up_in": upc,
            "refs_in": np.ascontiguousarray(refs[c]),
            "vee_in": vee,
            "ro1_in": ro1,
            "off_in": np.full((128, 1), OFF, np.float32),
            "ident_in": ident,
            "antid_in": antid,
        })
    return in_maps


def get_nc():
    if "nc" not in _cache:
        _cache["nc"] = _build_bass()
    return _cache["nc"]


def reduce_results(results):
    total = 0.0
    for r in results:
        total += float(r["out"].astype(np.float64).sum())
    return np.float32(total / (60.0 * B * W * W))


def kernel(up, left, right):
    nc = get_nc()
    in_maps = make_in_maps(up, left, right)
    res = run_bass_kernel_spmd(nc, in_maps, core_ids=list(range(NCORES)))
    return reduce_results(res.results)

